# revision 9
# baseline (speedup 1.0000x reference)
"""Cross-attention kernel for Trainium2, distributed over 8 NeuronCores.

Sharding: batch x head parallel. Cores 0-3 handle batch 0, cores 4-7 batch 1.
Within a team of 4, core r handles heads 4r..4r+3 (channel slice 256r..256r+256).

Per core:
  - KV projection for its 256 k-channels + 256 v-channels (tensor parallel,
    contraction over full D with host-pretransposed context/W_kv)
  - k LayerNorm: partial (mean, E[x^2]) per row via bn_stats + 16KB AllReduce
    within team
  - q LayerNorm: full-row stats from x[b], normalize only its channel slice
  - attention for its 4 heads, computed transposed (simT[j,i] = k.q) with
    softmax denominators from an appended ones-column in v (no max
    subtraction: |sim*scale| <= ~6 for this problem, exp stays in fp32 range).
    Sim tiles are computed in pairs so each Exp covers [128, 1024].
  - output projection is row-parallel over channels: each core projects its
    own 256 channels for all rows of a 512-row block (partial y, fp16, with
    b_out/4 added so the team sum carries the bias exactly once), then a
    per-block ReduceScatter sums within the team and leaves each core the
    128-row shard matching its team rank. Host assembles 8 cores x 4 blocks.
"""

import numpy as np

import concourse.bass as bass
import concourse.mybir as mybir
import concourse.tile as tile
from concourse import bacc
from concourse.bass_utils import run_bass_kernel_spmd
from concourse.masks import make_identity

B, NQ, NK, D, H, DH = 2, 2048, 2048, 1024, 16, 64
NCORES = 8
TEAM = 4
HPC = 4            # heads per core
DSL = HPC * DH     # 256: per-core channel slice
EPS = 1e-6
SCALE = DH ** -0.5
GROUPS = [[0, 1, 2, 3], [4, 5, 6, 7]]
FP32 = mybir.dt.float32
FP32R = mybir.dt.float32r
FP16 = mybir.dt.float16
NT = NQ // 128     # 16 row tiles
KC = D // 128      # 8 contraction chunks (kv projection)
KO = DSL // 128    # 2 contraction chunks (out projection)
NBLK = 4           # 512-row output blocks
VW = DH + 2        # 66: v tile width (v, ones, zero)

_CACHE: dict = {}
MOCK_COLL = False  # replace collectives with local DMA (for TimelineSim)


def _bcast_ap(t, parts):
    ap = t.ap() if hasattr(t, "ap") and not isinstance(t, bass.AP) else t
    return bass.AP(tensor=ap.tensor, offset=ap.offset,
                   ap=[[0, parts]] + list(ap.ap))


def _build():
    nc = bacc.Bacc("TRN2", target_bir_lowering=False, debug=False,
                   num_devices=NCORES)
    x_b = nc.declare_dram_parameter("x_b", [NQ, D], FP32, isOutput=False)
    ctxT = nc.declare_dram_parameter("ctxT", [D, NK], FP32, isOutput=False)
    wkvT = nc.declare_dram_parameter("wkvT", [D, 2 * DSL], FP32, isOutput=False)
    woutT = nc.declare_dram_parameter("woutT", [DSL, D], FP32, isOutput=False)
    bout = nc.declare_dram_parameter("bout", [D], FP32, isOutput=False)
    gq_s = nc.declare_dram_parameter("gq_s", [DSL], FP32, isOutput=False)
    bq_s = nc.declare_dram_parameter("bq_s", [DSL], FP32, isOutput=False)
    gk_s = nc.declare_dram_parameter("gk_s", [DSL], FP32, isOutput=False)
    bk_s = nc.declare_dram_parameter("bk_s", [DSL], FP32, isOutput=False)
    y_own = nc.declare_dram_parameter("y_own", [NBLK, 128, D], FP16,
                                      isOutput=True)

    stats_dram = nc.dram_tensor("stats_dram", [128, 2 * NT], FP32)
    statsr_dram = nc.dram_tensor("statsr_dram", [128, 2 * NT], FP32)
    ypart = [nc.dram_tensor(f"ypart{i}", [4 * 128, D], FP16)
             for i in range(NBLK)]
    yr_dram = nc.dram_tensor("yr_dram", [NBLK, 128, D], FP16)

    ctxT_r = ctxT.ap().rearrange("(k p) m -> p k m", p=128)    # [128, 8, NK]
    wkvT_r = wkvT.ap().rearrange("(k p) n -> p k n", p=128)    # [128, 8, 512]
    woutT_r = woutT.ap().rearrange("(k p) n -> p k n", p=128)  # [128, 2, D]

    with tile.TileContext(nc) as tc:
        with (
            tc.tile_pool(name="singles", bufs=1) as singles,
            tc.tile_pool(name="ld", bufs=3) as ld,
            tc.tile_pool(name="work", bufs=3) as work,
            tc.tile_pool(name="pssim", bufs=2, space="PSUM") as pssim,
            tc.tile_pool(name="psaux", bufs=2, space="PSUM") as psaux,
            tc.tile_pool(name="psout", bufs=2, space="PSUM") as psout,
        ):
            # --- persistent sbuf ---
            ident = singles.tile([128, 128], FP32)
            make_identity(nc, ident)
            eps_sb = singles.tile([128, 1], FP32)
            nc.vector.memset(eps_sb, EPS)

            def _col_ap(param, cb):
                ap = param.ap()
                return bass.AP(tensor=ap.tensor, offset=128 * cb,
                               ap=[[1, 128], [1, 1]])

            gqT = [singles.tile([128, 1], FP32, name=f"gqT{cb}") for cb in range(2)]
            bqT = [singles.tile([128, 1], FP32, name=f"bqT{cb}") for cb in range(2)]
            gkT = [singles.tile([128, 1], FP32, name=f"gkT{cb}") for cb in range(2)]
            bkT = [singles.tile([128, 1], FP32, name=f"bkT{cb}") for cb in range(2)]
            for cb in range(2):
                nc.sync.dma_start(out=gqT[cb], in_=_col_ap(gq_s, cb))
                nc.sync.dma_start(out=bqT[cb], in_=_col_ap(bq_s, cb))
                nc.sync.dma_start(out=gkT[cb], in_=_col_ap(gk_s, cb))
                nc.sync.dma_start(out=bkT[cb], in_=_col_ap(bk_s, cb))
            wkv_sb = singles.tile([128, KC, 2 * DSL], FP32R)
            nc.sync.dma_start(out=wkv_sb, in_=wkvT_r.bitcast(FP32R))

            k_nat = singles.tile([128, NT, DSL], FP32)
            vh_sb = singles.tile([128, NT, HPC, VW], FP32R)
            nc.vector.memset(vh_sb[:, :, :, DH:DH + 1].bitcast(FP32), 1.0)
            nc.vector.memset(vh_sb[:, :, :, DH + 1:DH + 2].bitcast(FP32), 0.0)
            qT_sb = [singles.tile([128, NT, 128], FP32R, tag=f"qT{cb}",
                                  name=f"qT{cb}") for cb in range(2)]
            kT_sb = [singles.tile([128, NT, 128], FP32R, tag=f"kT{cb}",
                                  name=f"kT{cb}") for cb in range(2)]
            aoT_sb = [singles.tile([128, NQ], FP32R, tag=f"aoT{cb}",
                                   name=f"aoT{cb}") for cb in range(2)]
            stats_sb = singles.tile([128, NT, 2], FP32)
            statsr_sb = singles.tile([128, NT, 2], FP32)
            mean_all = singles.tile([128, NT], FP32)
            var_all = singles.tile([128, NT], FP32)
            rstd_all = singles.tile([128, NT], FP32)
            wout_sb = singles.tile([128, KO, D], FP32R)
            bout_b = singles.tile([128, D], FP32)

            # --- stage A+C interleaved: kv-proj, k stats, q LN, q transpose ---
            for t in range(NT):
                # A: kv projection for NK row tile t (uses half a sim pair tile)
                ctx_sb = ld.tile([128, KC, 128], FP32R, tag="ctx")
                nc.sync.dma_start(out=ctx_sb,
                                  in_=ctxT_r[:, :, 128 * t:128 * (t + 1)]
                                  .bitcast(FP32R))
                kv_ps = pssim.tile([128, 1024], FP32, tag="sim", name="kv_ps")
                for kk in range(KC):
                    nc.tensor.matmul(kv_ps[:, 0:2 * DSL], lhsT=ctx_sb[:, kk, :],
                                     rhs=wkv_sb[:, kk, :],
                                     start=(kk == 0), stop=(kk == KC - 1))
                nc.vector.tensor_copy(k_nat[:, t, :], kv_ps[:, 0:DSL])
                nc.vector.tensor_copy(
                    vh_sb[:, t, :, 0:DH], kv_ps[:, DSL:2 * DSL])
                # k partial stats (pre-norm): mean and E[x^2] over own slice
                bnk = work.tile([128, 1, 6], FP32, tag="bnk")
                nc.vector.bn_stats(out=bnk[:, 0, :], in_=kv_ps[:, 0:DSL])
                mvk = work.tile([128, 2], FP32, tag="mvk")
                nc.vector.bn_aggr(out=mvk, in_=bnk)
                nc.vector.tensor_copy(stats_sb[:, t, 0:1], mvk[:, 0:1])
                m2k = work.tile([128, 1], FP32, tag="m2k")
                nc.vector.tensor_mul(m2k, mvk[:, 0:1], mvk[:, 0:1])
                nc.vector.tensor_add(stats_sb[:, t, 1:2], mvk[:, 1:2], m2k)

                # C: q LayerNorm for NQ row tile t
                x_sb = ld.tile([128, D], FP32, tag="x")
                nc.gpsimd.dma_start(out=x_sb, in_=x_b[128 * t:128 * (t + 1), :])
                bn = work.tile([128, 2, 6], FP32, tag="bn")
                nc.vector.bn_stats(out=bn[:, 0, :], in_=x_sb[:, 0:512])
                nc.vector.bn_stats(out=bn[:, 1, :], in_=x_sb[:, 512:1024])
                mv = work.tile([128, 2], FP32, tag="mv")
                nc.vector.bn_aggr(out=mv, in_=bn)
                sdev = work.tile([128, 1], FP32, tag="sdev")
                nc.scalar.activation(sdev, mv[:, 1:2],
                                     mybir.ActivationFunctionType.Sqrt,
                                     bias=eps_sb)
                rq = work.tile([128, 1], FP32, tag="rq")
                nc.vector.reciprocal(rq, sdev)
                q_nat = work.tile([128, DSL], FP32, tag="qn")
                nc.vector.tensor_scalar(out=q_nat, in0=x_sb[:, 0:DSL],
                                        scalar1=mv[:, 0:1], scalar2=rq,
                                        op0=mybir.AluOpType.subtract,
                                        op1=mybir.AluOpType.mult)
                for cb in range(2):
                    tp_ps = psaux.tile([128, 512], FP32, tag="aux", name="tp_ps")
                    nc.tensor.transpose(tp_ps[:, 0:128],
                                        q_nat[:, 128 * cb:128 * (cb + 1)],
                                        ident)
                    nc.vector.tensor_scalar(out=qT_sb[cb][:, t, :],
                                            in0=tp_ps[:, 0:128],
                                            scalar1=gqT[cb], scalar2=bqT[cb],
                                            op0=mybir.AluOpType.mult,
                                            op1=mybir.AluOpType.add)

            # weights needed later; keep them off the early DMA critical path
            nc.sync.dma_start(out=wout_sb, in_=woutT_r.bitcast(FP32R))
            nc.sync.dma_start(out=bout_b, in_=_bcast_ap(bout, 128))

            # --- stage B: AllReduce k stats within team ---
            nc.sync.dma_start(out=stats_dram[:, :],
                              in_=stats_sb.rearrange("p t s -> p (t s)"))
            if MOCK_COLL:
                nc.sync.dma_start(out=statsr_dram[:, :], in_=stats_dram[:, :])
            else:
                nc.gpsimd.collective_compute(
                    "AllReduce", mybir.AluOpType.add, replica_groups=GROUPS,
                    ins=[stats_dram.ap().opt()], outs=[statsr_dram.ap().opt()])
            nc.sync.dma_start(out=statsr_sb.rearrange("p t s -> p (t s)"),
                              in_=statsr_dram[:, :])
            nc.vector.tensor_scalar_mul(mean_all, in0=statsr_sb[:, :, 0],
                                        scalar1=1.0 / TEAM)
            nc.vector.tensor_scalar_mul(var_all, in0=statsr_sb[:, :, 1],
                                        scalar1=1.0 / TEAM)
            m2 = work.tile([128, NT], FP32, tag="m2")
            nc.vector.tensor_mul(m2, mean_all, mean_all)
            nc.vector.tensor_sub(var_all, var_all, m2)
            nc.scalar.activation(var_all, var_all,
                                 mybir.ActivationFunctionType.Sqrt, bias=eps_sb)
            nc.vector.reciprocal(rstd_all, var_all)

            # --- stage D+E: k LN apply + k transpose ---
            for t in range(NT):
                nc.vector.tensor_scalar(out=k_nat[:, t, :], in0=k_nat[:, t, :],
                                        scalar1=mean_all[:, t:t + 1],
                                        scalar2=rstd_all[:, t:t + 1],
                                        op0=mybir.AluOpType.subtract,
                                        op1=mybir.AluOpType.mult)
                for cb in range(2):
                    tp_ps = psaux.tile([128, 512], FP32, tag="aux", name="tp_ps")
                    nc.tensor.transpose(tp_ps[:, 0:128],
                                        k_nat[:, t, 128 * cb:128 * (cb + 1)],
                                        ident)
                    nc.vector.tensor_scalar(out=kT_sb[cb][:, t, :],
                                            in0=tp_ps[:, 0:128],
                                            scalar1=gkT[cb], scalar2=bkT[cb],
                                            op0=mybir.AluOpType.mult,
                                            op1=mybir.AluOpType.add)

            # --- stage F+H fused: attention; the out-projection for block
            # b is row-parallel over this core's 256 channels, emitted in
            # 128-row chunks interleaved into block b+1's attention so the
            # in-order PE queue never starves the Exp pipeline ---
            def outproj_chunk(iblk, ii):
                y_sb = work.tile([128, D], FP16, tag="y", bufs=2, name="y_sb")
                for eb in range(2):
                    y_ps = psaux.tile([128, 512], FP32, tag="aux", name="y_ps")
                    for kk in range(KO):
                        nc.tensor.matmul(
                            y_ps,
                            lhsT=aoT_sb[kk][:, 512 * iblk + 128 * ii:
                                            512 * iblk + 128 * (ii + 1)],
                            rhs=wout_sb[:, kk, 512 * eb:512 * (eb + 1)],
                            start=(kk == 0), stop=(kk == KO - 1))
                    nc.vector.tensor_add(y_sb[:, 512 * eb:512 * (eb + 1)],
                                         y_ps,
                                         bout_b[:, 512 * eb:512 * (eb + 1)])
                nc.sync.dma_start(
                    out=ypart[iblk][128 * ii:128 * (ii + 1), :], in_=y_sb)

            def reduce_scatter(iblk):
                if MOCK_COLL:
                    nc.sync.dma_start(out=yr_dram.ap()[iblk],
                                      in_=ypart[iblk].ap()[0:128, :])
                else:
                    nc.gpsimd.collective_compute(
                        "ReduceScatter", mybir.AluOpType.add,
                        replica_groups=GROUPS,
                        ins=[ypart[iblk].ap().opt()],
                        outs=[yr_dram.ap()[iblk].opt()])
                nc.sync.dma_start(out=y_own[iblk], in_=yr_dram.ap()[iblk])

            for iblk in range(NBLK):
                for h in range(HPC):
                    cb, hh = h // 2, h % 2
                    khT = kT_sb[cb][64 * hh:64 * (hh + 1), :, :]
                    qhT = qT_sb[cb][64 * hh:64 * (hh + 1), :, :]
                    oT_ps = psout.tile([VW, 512], FP32, tag="oT")
                    for jp in range(NT // 2):
                        s_ps = pssim.tile([128, 1024], FP32, tag="sim")
                        e_sb = work.tile([128, 1024], FP32R, tag="exp", bufs=3)
                        for half in range(2):
                            j = 2 * jp + half
                            nc.tensor.matmul(
                                s_ps[:, 512 * half:512 * (half + 1)],
                                lhsT=khT[:, j, :],
                                rhs=qhT[:, 4 * iblk:4 * (iblk + 1), :],
                                start=True, stop=True)
                        nc.scalar.activation(e_sb, s_ps,
                                             mybir.ActivationFunctionType.Exp,
                                             scale=SCALE)
                        for half in range(2):
                            j = 2 * jp + half
                            nc.tensor.matmul(
                                oT_ps, lhsT=vh_sb[:, j, h, :],
                                rhs=e_sb[:, 512 * half:512 * (half + 1)],
                                start=(j == 0), stop=(j == NT - 1))
                    # normalize: row DH of oT_ps holds the softmax denominators
                    csr = work.tile([1, 512], FP32, tag="csr", bufs=2)
                    nc.vector.reciprocal(csr, oT_ps[DH:DH + 1, :])
                    csb = work.tile([64, 512], FP32, tag="csb", bufs=2)
                    nc.gpsimd.partition_broadcast(csb, csr)
                    nc.vector.tensor_mul(
                        aoT_sb[cb][64 * hh:64 * (hh + 1),
                                   512 * iblk:512 * (iblk + 1)],
                        oT_ps[0:DH, :], csb)
                    if iblk > 0:
                        outproj_chunk(iblk - 1, h)
                if iblk > 0:
                    reduce_scatter(iblk - 1)
            for ii in range(4):
                outproj_chunk(NBLK - 1, ii)
            reduce_scatter(NBLK - 1)

    nc.finalize()
    return nc


def kernel(x, context, gq, bq, gk, bk, W_kv, W_out, b_out):
    x = np.asarray(x, dtype=np.float32)
    context = np.asarray(context, dtype=np.float32)
    gq = np.asarray(gq, dtype=np.float32)
    bq = np.asarray(bq, dtype=np.float32)
    gk = np.asarray(gk, dtype=np.float32)
    bk = np.asarray(bk, dtype=np.float32)
    W_kv = np.asarray(W_kv, dtype=np.float32)
    W_out = np.asarray(W_out, dtype=np.float32)
    b_out = np.asarray(b_out, dtype=np.float32)

    if "nc" not in _CACHE:
        _CACHE["nc"] = _build()
    nc = _CACHE["nc"]

    Wk, Wv = W_kv[:D], W_kv[D:]
    in_maps = []
    for c in range(NCORES):
        b, r = c // TEAM, c % TEAM
        sl = slice(DSL * r, DSL * (r + 1))
        wkvT_c = np.ascontiguousarray(
            np.concatenate([Wk[sl], Wv[sl]], axis=0).T)
        in_maps.append({
            # roll channels so this core's q slice sits at cols 0:DSL
            # (LayerNorm full-row stats are permutation invariant)
            "x_b": np.ascontiguousarray(np.roll(x[b], -DSL * r, axis=1)),
            "ctxT": np.ascontiguousarray(context[b].T),
            "wkvT": wkvT_c,
            "woutT": np.ascontiguousarray(W_out.T[sl]),
            "bout": b_out / 4.0,
            "gq_s": np.ascontiguousarray(gq[sl]),
            "bq_s": np.ascontiguousarray(bq[sl]),
            "gk_s": np.ascontiguousarray(gk[sl]),
            "bk_s": np.ascontiguousarray(bk[sl]),
        })

    _CACHE["in_maps"] = in_maps
    try:
        res = run_bass_kernel_spmd(nc, in_maps, list(range(NCORES))).results
    except Exception:
        # transient runtime failures (device wedged from a prior run) --
        # one retry typically succeeds
        res = run_bass_kernel_spmd(nc, in_maps, list(range(NCORES))).results
    y = np.empty((B, NQ, D), dtype=np.float32)
    for c in range(NCORES):
        b, r = c // TEAM, c % TEAM
        for t in range(NBLK):
            y[b, 512 * t + 128 * r:512 * t + 128 * (r + 1), :] = \
                np.asarray(res[c]["y_own"][t], dtype=np.float32)
    return y


# revision 17
# speedup vs baseline: 1.0484x; 1.0484x over previous
"""Cross-attention kernel for Trainium2, distributed over 8 NeuronCores.

Sharding: batch x head parallel. Cores 0-3 handle batch 0, cores 4-7 batch 1.
Within a team of 4, core r handles heads 4r..4r+3 (channel slice 256r..256r+256).

Per core:
  - KV projection for its 256 k-channels + 256 v-channels (tensor parallel,
    contraction over full D with host-pretransposed context/W_kv)
  - k LayerNorm: partial (mean, E[x^2]) per row via bn_stats + 16KB AllReduce
    within team
  - q LayerNorm: full-row stats from x[b], normalize only its channel slice
  - attention for its 4 heads, computed transposed (simT[j,i] = k.q) with
    softmax denominators from an appended ones-column in v (no max
    subtraction: |sim*scale| <= ~6 for this problem, exp stays in fp32 range).
    Sim tiles are computed in pairs so each Exp covers [128, 1024].
  - output projection is row-parallel over channels: each core projects its
    own 256 channels for all rows of a 512-row block (partial y, fp16, with
    b_out/4 added so the team sum carries the bias exactly once), then a
    per-block ReduceScatter sums within the team and leaves each core the
    128-row shard matching its team rank. Host assembles 8 cores x 4 blocks.
"""

import numpy as np

import concourse.bass as bass
import concourse.mybir as mybir
import concourse.tile as tile
from concourse import bacc
from concourse.bass_utils import run_bass_kernel_spmd
from concourse.masks import make_identity

B, NQ, NK, D, H, DH = 2, 2048, 2048, 1024, 16, 64
NCORES = 8
TEAM = 4
HPC = 4            # heads per core
DSL = HPC * DH     # 256: per-core channel slice
EPS = 1e-6
SCALE = DH ** -0.5
GROUPS = [[0, 1, 2, 3], [4, 5, 6, 7]]
FP32 = mybir.dt.float32
FP32R = mybir.dt.float32r
FP16 = mybir.dt.float16
BF16 = mybir.dt.bfloat16
NT = NQ // 128     # 16 row tiles
KC = D // 128      # 8 contraction chunks (kv projection)
KO = DSL // 128    # 2 contraction chunks (out projection)
NBLK = 4           # 512-row output blocks
VW = DH + 2        # 66: v tile width (v, ones, zero)

_CACHE: dict = {}
MOCK_COLL = False  # replace collectives with local DMA (for TimelineSim)


def _bcast_ap(t, parts):
    ap = t.ap() if hasattr(t, "ap") and not isinstance(t, bass.AP) else t
    return bass.AP(tensor=ap.tensor, offset=ap.offset,
                   ap=[[0, parts]] + list(ap.ap))


def _build():
    nc = bacc.Bacc("TRN2", target_bir_lowering=False, debug=False,
                   num_devices=NCORES)
    x_b = nc.declare_dram_parameter("x_b", [NQ, D], BF16, isOutput=False)
    ctxT = nc.declare_dram_parameter("ctxT", [D, NK], BF16, isOutput=False)
    wkvT = nc.declare_dram_parameter("wkvT", [D, 2 * DSL], BF16, isOutput=False)
    woutT = nc.declare_dram_parameter("woutT", [DSL, D], FP32, isOutput=False)
    bout = nc.declare_dram_parameter("bout", [D], FP32, isOutput=False)
    gq_s = nc.declare_dram_parameter("gq_s", [DSL], FP32, isOutput=False)
    bq_s = nc.declare_dram_parameter("bq_s", [DSL], FP32, isOutput=False)
    gk_s = nc.declare_dram_parameter("gk_s", [DSL], FP32, isOutput=False)
    bk_s = nc.declare_dram_parameter("bk_s", [DSL], FP32, isOutput=False)
    y_own = nc.declare_dram_parameter("y_own", [NBLK, 128, D], FP16,
                                      isOutput=True)

    stats_dram = nc.dram_tensor("stats_dram", [128, 2 * NT], FP32)
    statsr_dram = nc.dram_tensor("statsr_dram", [128, 2 * NT], FP32)
    ypart = [nc.dram_tensor(f"ypart{i}", [4 * 128, D], FP16)
             for i in range(NBLK)]
    yr_dram = nc.dram_tensor("yr_dram", [NBLK, 128, D], FP16)

    ctxT_r = ctxT.ap().rearrange("(k p) m -> p k m", p=128)    # [128, 8, NK]
    wkvT_r = wkvT.ap().rearrange("(k p) n -> p k n", p=128)    # [128, 8, 512]
    woutT_r = woutT.ap().rearrange("(k p) n -> p k n", p=128)  # [128, 2, D]

    with tile.TileContext(nc) as tc:
        with (
            tc.tile_pool(name="singles", bufs=1) as singles,
            tc.tile_pool(name="ld", bufs=3) as ld,
            tc.tile_pool(name="work", bufs=3) as work,
            tc.tile_pool(name="pssim", bufs=2, space="PSUM") as pssim,
            tc.tile_pool(name="psaux", bufs=2, space="PSUM") as psaux,
            tc.tile_pool(name="psout", bufs=2, space="PSUM") as psout,
        ):
            # --- persistent sbuf ---
            ident = singles.tile([128, 128], FP32)
            make_identity(nc, ident)
            eps_sb = singles.tile([128, 1], FP32)
            nc.vector.memset(eps_sb, EPS)

            def _col_ap(param, cb):
                ap = param.ap()
                return bass.AP(tensor=ap.tensor, offset=128 * cb,
                               ap=[[1, 128], [1, 1]])

            gqT = [singles.tile([128, 1], FP32, name=f"gqT{cb}") for cb in range(2)]
            bqT = [singles.tile([128, 1], FP32, name=f"bqT{cb}") for cb in range(2)]
            gkT = [singles.tile([128, 1], FP32, name=f"gkT{cb}") for cb in range(2)]
            bkT = [singles.tile([128, 1], FP32, name=f"bkT{cb}") for cb in range(2)]
            for cb in range(2):
                nc.gpsimd.dma_start(out=gqT[cb], in_=_col_ap(gq_s, cb))
                nc.gpsimd.dma_start(out=bqT[cb], in_=_col_ap(bq_s, cb))
                nc.gpsimd.dma_start(out=gkT[cb], in_=_col_ap(gk_s, cb))
                nc.gpsimd.dma_start(out=bkT[cb], in_=_col_ap(bk_s, cb))
            wkv_sb = singles.tile([128, KC, 2 * DSL], BF16)
            nc.gpsimd.dma_start(out=wkv_sb, in_=wkvT_r)

            k_nat = singles.tile([128, NT, DSL], FP32)
            vh_sb = singles.tile([128, NT, HPC, VW], FP32R)
            nc.vector.memset(vh_sb[:, :, :, DH:DH + 1].bitcast(FP32), 1.0)
            nc.vector.memset(vh_sb[:, :, :, DH + 1:DH + 2].bitcast(FP32), 0.0)
            qT_sb = [singles.tile([128, NT, 128], FP32R, tag=f"qT{cb}",
                                  name=f"qT{cb}") for cb in range(2)]
            kT_sb = [singles.tile([128, NT, 128], FP32R, tag=f"kT{cb}",
                                  name=f"kT{cb}") for cb in range(2)]
            aoT_sb = [singles.tile([128, NQ], FP32R, tag=f"aoT{cb}",
                                   name=f"aoT{cb}") for cb in range(2)]
            stats_sb = singles.tile([128, NT, 2], FP32)
            statsr_sb = singles.tile([128, NT, 2], FP32)
            mean_all = singles.tile([128, NT], FP32)
            var_all = singles.tile([128, NT], FP32)
            rstd_all = singles.tile([128, NT], FP32)
            wout_sb = singles.tile([128, KO, D], FP32R)
            bout_b = singles.tile([128, D], FP32)

            # --- stage A+C interleaved: kv-proj, k stats, q LN, q transpose.
            # ctx streams in 2MB tiles on the sync HWDGE queue; x streams on
            # the Activation HWDGE queue (idle until attention); wkv/params
            # went to the Pool SWDGE queue — three parallel DMA channels ---
            x_sb = None
            for t in range(NT):
                sub = t % 4
                if sub == 0:
                    ctx_sb = ld.tile([128, KC, 512], BF16, tag="ctx", bufs=2)
                    nc.sync.dma_start(out=ctx_sb,
                                      in_=ctxT_r[:, :, 512 * (t // 4):
                                                 512 * (t // 4 + 1)])
                if t % 2 == 0:
                    x_sb = ld.tile([128, 2, D], BF16, tag="x")
                    nc.gpsimd.dma_start(
                        out=x_sb,
                        in_=x_b.ap()[128 * t:128 * (t + 2), :]
                        .rearrange("(s p) d -> p s d", p=128))
                # A: kv projection for NK row tile t (uses half a sim pair tile)
                kv_ps = pssim.tile([128, 1024], FP32, tag="sim", name="kv_ps")
                for kk in range(KC):
                    nc.tensor.matmul(kv_ps[:, 0:2 * DSL],
                                     lhsT=ctx_sb[:, kk,
                                                 128 * sub:128 * (sub + 1)],
                                     rhs=wkv_sb[:, kk, :],
                                     start=(kk == 0), stop=(kk == KC - 1))
                nc.scalar.copy(k_nat[:, t, :], kv_ps[:, 0:DSL])
                for h in range(HPC):
                    nc.scalar.copy(vh_sb[:, t, h, 0:DH],
                                   kv_ps[:, DSL + DH * h:DSL + DH * (h + 1)])
                # k partial stats (pre-norm): mean and E[x^2] over own slice
                bnk = work.tile([128, 1, 6], FP32, tag="bnk")
                nc.vector.bn_stats(out=bnk[:, 0, :], in_=k_nat[:, t, :])
                mvk = work.tile([128, 2], FP32, tag="mvk")
                nc.vector.bn_aggr(out=mvk, in_=bnk)
                nc.vector.tensor_copy(stats_sb[:, t, 0:1], mvk[:, 0:1])
                m2k = work.tile([128, 1], FP32, tag="m2k")
                nc.vector.tensor_mul(m2k, mvk[:, 0:1], mvk[:, 0:1])
                nc.vector.tensor_add(stats_sb[:, t, 1:2], mvk[:, 1:2], m2k)

                # C: q LayerNorm for NQ row tile t
                xs = x_sb[:, t % 2, :]
                bn = work.tile([128, 2, 6], FP32, tag="bn")
                nc.vector.bn_stats(out=bn[:, 0, :], in_=xs[:, 0:512])
                nc.vector.bn_stats(out=bn[:, 1, :], in_=xs[:, 512:1024])
                mv = work.tile([128, 2], FP32, tag="mv")
                nc.vector.bn_aggr(out=mv, in_=bn)
                sdev = work.tile([128, 1], FP32, tag="sdev")
                nc.scalar.activation(sdev, mv[:, 1:2],
                                     mybir.ActivationFunctionType.Sqrt,
                                     bias=eps_sb)
                rq = work.tile([128, 1], FP32, tag="rq")
                nc.vector.reciprocal(rq, sdev)
                q_nat = work.tile([128, DSL], FP32, tag="qn")
                nc.vector.tensor_scalar(out=q_nat, in0=xs[:, 0:DSL],
                                        scalar1=mv[:, 0:1], scalar2=rq,
                                        op0=mybir.AluOpType.subtract,
                                        op1=mybir.AluOpType.mult)
                for cb in range(2):
                    tp_ps = psaux.tile([128, 512], FP32, tag="aux", name="tp_ps")
                    nc.tensor.transpose(tp_ps[:, 0:128],
                                        q_nat[:, 128 * cb:128 * (cb + 1)],
                                        ident)
                    if cb == 0:
                        nc.scalar.activation(
                            qT_sb[cb][:, t, :], tp_ps[:, 0:128],
                            mybir.ActivationFunctionType.Identity,
                            scale=gqT[cb], bias=bqT[cb])
                    else:
                        nc.vector.tensor_scalar(
                            out=qT_sb[cb][:, t, :], in0=tp_ps[:, 0:128],
                            scalar1=gqT[cb], scalar2=bqT[cb],
                            op0=mybir.AluOpType.mult,
                            op1=mybir.AluOpType.add)

            # weights needed later; keep them off the early DMA critical path
            nc.sync.dma_start(out=wout_sb, in_=woutT_r.bitcast(FP32R))
            nc.sync.dma_start(out=bout_b, in_=_bcast_ap(bout, 128))

            # --- stage B: AllReduce k stats within team ---
            nc.sync.dma_start(out=stats_dram[:, :],
                              in_=stats_sb.rearrange("p t s -> p (t s)"))
            if MOCK_COLL:
                nc.sync.dma_start(out=statsr_dram[:, :], in_=stats_dram[:, :])
            else:
                nc.gpsimd.collective_compute(
                    "AllReduce", mybir.AluOpType.add, replica_groups=GROUPS,
                    ins=[stats_dram.ap().opt()], outs=[statsr_dram.ap().opt()])
            nc.sync.dma_start(out=statsr_sb.rearrange("p t s -> p (t s)"),
                              in_=statsr_dram[:, :])
            nc.vector.tensor_scalar_mul(mean_all, in0=statsr_sb[:, :, 0],
                                        scalar1=1.0 / TEAM)
            nc.vector.tensor_scalar_mul(var_all, in0=statsr_sb[:, :, 1],
                                        scalar1=1.0 / TEAM)
            m2 = work.tile([128, NT], FP32, tag="m2")
            nc.vector.tensor_mul(m2, mean_all, mean_all)
            nc.vector.tensor_sub(var_all, var_all, m2)
            nc.scalar.activation(var_all, var_all,
                                 mybir.ActivationFunctionType.Sqrt, bias=eps_sb)
            nc.vector.reciprocal(rstd_all, var_all)

            # --- stage D+E: k LN apply + k transpose ---
            for t in range(NT):
                nc.vector.tensor_scalar(out=k_nat[:, t, :], in0=k_nat[:, t, :],
                                        scalar1=mean_all[:, t:t + 1],
                                        scalar2=rstd_all[:, t:t + 1],
                                        op0=mybir.AluOpType.subtract,
                                        op1=mybir.AluOpType.mult)
                for cb in range(2):
                    tp_ps = psaux.tile([128, 512], FP32, tag="aux", name="tp_ps")
                    nc.tensor.transpose(tp_ps[:, 0:128],
                                        k_nat[:, t, 128 * cb:128 * (cb + 1)],
                                        ident)
                    if cb == 0:
                        nc.scalar.activation(
                            kT_sb[cb][:, t, :], tp_ps[:, 0:128],
                            mybir.ActivationFunctionType.Identity,
                            scale=gkT[cb], bias=bkT[cb])
                    else:
                        nc.vector.tensor_scalar(
                            out=kT_sb[cb][:, t, :], in0=tp_ps[:, 0:128],
                            scalar1=gkT[cb], scalar2=bkT[cb],
                            op0=mybir.AluOpType.mult,
                            op1=mybir.AluOpType.add)

            # --- stage F+H fused: attention; the out-projection for block
            # b is row-parallel over this core's 256 channels, emitted in
            # 128-row chunks interleaved into block b+1's attention so the
            # in-order PE queue never starves the Exp pipeline ---
            def outproj_chunk(iblk, ii):
                y_sb = work.tile([128, D], FP16, tag="y", bufs=2, name="y_sb")
                for eb in range(2):
                    y_ps = psaux.tile([128, 512], FP32, tag="aux", name="y_ps")
                    for kk in range(KO):
                        nc.tensor.matmul(
                            y_ps,
                            lhsT=aoT_sb[kk][:, 512 * iblk + 128 * ii:
                                            512 * iblk + 128 * (ii + 1)],
                            rhs=wout_sb[:, kk, 512 * eb:512 * (eb + 1)],
                            start=(kk == 0), stop=(kk == KO - 1))
                    nc.vector.tensor_add(y_sb[:, 512 * eb:512 * (eb + 1)],
                                         y_ps,
                                         bout_b[:, 512 * eb:512 * (eb + 1)])
                nc.sync.dma_start(
                    out=ypart[iblk][128 * ii:128 * (ii + 1), :], in_=y_sb)

            def reduce_scatter(iblk):
                if MOCK_COLL:
                    nc.sync.dma_start(out=yr_dram.ap()[iblk],
                                      in_=ypart[iblk].ap()[0:128, :])
                else:
                    nc.gpsimd.collective_compute(
                        "ReduceScatter", mybir.AluOpType.add,
                        replica_groups=GROUPS,
                        ins=[ypart[iblk].ap().opt()],
                        outs=[yr_dram.ap()[iblk].opt()])
                nc.gpsimd.dma_start(out=y_own[iblk], in_=yr_dram.ap()[iblk])

            for iblk in range(NBLK):
                for h in range(HPC):
                    cb, hh = h // 2, h % 2
                    khT = kT_sb[cb][64 * hh:64 * (hh + 1), :, :]
                    qhT = qT_sb[cb][64 * hh:64 * (hh + 1), :, :]
                    oT_ps = psout.tile([VW, 512], FP32, tag="oT")
                    for jp in range(NT // 2):
                        s_ps = pssim.tile([128, 1024], FP32, tag="sim")
                        e_sb = work.tile([128, 1024], FP32R, tag="exp", bufs=3)
                        for half in range(2):
                            j = 2 * jp + half
                            nc.tensor.matmul(
                                s_ps[:, 512 * half:512 * (half + 1)],
                                lhsT=khT[:, j, :],
                                rhs=qhT[:, 4 * iblk:4 * (iblk + 1), :],
                                start=True, stop=True)
                        nc.scalar.activation(e_sb, s_ps,
                                             mybir.ActivationFunctionType.Exp,
                                             scale=SCALE)
                        for half in range(2):
                            j = 2 * jp + half
                            nc.tensor.matmul(
                                oT_ps, lhsT=vh_sb[:, j, h, :],
                                rhs=e_sb[:, 512 * half:512 * (half + 1)],
                                start=(j == 0), stop=(j == NT - 1))
                    # normalize: row DH of oT_ps holds the softmax denominators
                    csr = work.tile([1, 512], FP32, tag="csr", bufs=2)
                    nc.vector.reciprocal(csr, oT_ps[DH:DH + 1, :])
                    csb = work.tile([64, 512], FP32, tag="csb", bufs=2)
                    nc.gpsimd.partition_broadcast(csb, csr)
                    nc.vector.tensor_mul(
                        aoT_sb[cb][64 * hh:64 * (hh + 1),
                                   512 * iblk:512 * (iblk + 1)],
                        oT_ps[0:DH, :], csb)
                    if iblk > 0:
                        outproj_chunk(iblk - 1, h)
                if iblk > 0:
                    reduce_scatter(iblk - 1)
            for ii in range(4):
                outproj_chunk(NBLK - 1, ii)
            reduce_scatter(NBLK - 1)

    nc.finalize()
    return nc


def kernel(x, context, gq, bq, gk, bk, W_kv, W_out, b_out):
    x = np.asarray(x, dtype=np.float32)
    context = np.asarray(context, dtype=np.float32)
    gq = np.asarray(gq, dtype=np.float32)
    bq = np.asarray(bq, dtype=np.float32)
    gk = np.asarray(gk, dtype=np.float32)
    bk = np.asarray(bk, dtype=np.float32)
    W_kv = np.asarray(W_kv, dtype=np.float32)
    W_out = np.asarray(W_out, dtype=np.float32)
    b_out = np.asarray(b_out, dtype=np.float32)

    if "nc" not in _CACHE:
        _CACHE["nc"] = _build()
    nc = _CACHE["nc"]

    Wk, Wv = W_kv[:D], W_kv[D:]
    in_maps = []
    for c in range(NCORES):
        b, r = c // TEAM, c % TEAM
        sl = slice(DSL * r, DSL * (r + 1))
        wkvT_c = np.ascontiguousarray(
            np.concatenate([Wk[sl], Wv[sl]], axis=0).T)
        import ml_dtypes
        bf16 = ml_dtypes.bfloat16
        in_maps.append({
            # roll channels so this core's q slice sits at cols 0:DSL
            # (LayerNorm full-row stats are permutation invariant)
            "x_b": np.ascontiguousarray(np.roll(x[b], -DSL * r, axis=1))
            .astype(bf16),
            "ctxT": np.ascontiguousarray(context[b].T).astype(bf16),
            "wkvT": wkvT_c.astype(bf16),
            "woutT": np.ascontiguousarray(W_out.T[sl]),
            "bout": b_out / 4.0,
            "gq_s": np.ascontiguousarray(gq[sl]),
            "bq_s": np.ascontiguousarray(bq[sl]),
            "gk_s": np.ascontiguousarray(gk[sl]),
            "bk_s": np.ascontiguousarray(bk[sl]),
        })

    _CACHE["in_maps"] = in_maps
    try:
        res = run_bass_kernel_spmd(nc, in_maps, list(range(NCORES))).results
    except Exception:
        # transient runtime failures (device wedged from a prior run) --
        # one retry typically succeeds
        res = run_bass_kernel_spmd(nc, in_maps, list(range(NCORES))).results
    y = np.empty((B, NQ, D), dtype=np.float32)
    for c in range(NCORES):
        b, r = c // TEAM, c % TEAM
        for t in range(NBLK):
            y[b, 512 * t + 128 * r:512 * t + 128 * (r + 1), :] = \
                np.asarray(res[c]["y_own"][t], dtype=np.float32)
    return y
